# revision 49
# baseline (speedup 1.0000x reference)
"""Differentiable palette quantization on 8 Trainium2 NeuronCores.

Math: for each image b, pixel x, palette p_k (k=64):
    w = softmax_k(-|x - p_k|^2 / T);  out = sum_k w_k p_k
Softmax is invariant to the per-pixel |x|^2 term, so the logit reduces to
    (2/T)*dot(x, p_k) + bias_k  with exact bias_k = -|p_k|^2/T supplied
through the ACT activation's per-partition bias operand.  The weighted
sum and the softmax denominator come from one matmul against
[palette | ones] (contraction over k = partitions).

Sharding: pure data parallel, 2 images per core.  A core's two images are
stacked on partitions (64+64 palette entries) and share the pixel stream
via a block-diagonal stationary matrix.  The dot matmul runs in bf16 with
an exact hi/lo fixup folded into the contraction dim (K=18 rows
[xh|xl|xh] against [ph|ph|pl]), so it costs the same PE streaming time as
a single bf16 pass.  (Plain fp32 matmul is 4 cyc/row and disables fast
weight load for the whole program; fp16 stationaries hang the PE.)

Pixels are processed as 128 strip-tasks (strip j = PE 32-row tile_position
row group, 512 pixels each), three per round into a [128, 1536] psum tile
(3 banks x 2 buffers) so the exp runs as one FD=1536 ACT op per round —
ACT is the bottleneck engine and runs back-to-back with zero gaps.
The e values are written as fp16 so the weighted-sum matmuls (lhsT =
e-block [128, 128], rhs = palW [128, 8]) get fast weight load.  Their
[128, 8] outputs accumulate in a 1-bank psum2 ([128, 480], 2 buffers);
when full, DVE computes numer * 1/denom and GPSIMD DMAs the [128, 180]
per-image block out.  Host reorders blocks back to image layout.
"""

import os
import sys

for _p in ("/opt/trn_rl_repo", os.path.expanduser("~/.axon_site/_ro/trn_rl_repo")):
    if os.path.isdir(_p) and _p not in sys.path:
        sys.path.insert(0, _p)

import numpy as np

import concourse.bass as bass
import concourse.tile as tile
from concourse import bacc, mybir
from concourse.bass_utils import run_bass_kernel_spmd

# problem constants (hardcoded per contract)
B, H, W, C, K = 16, 256, 256, 3, 64
NCORES = 8
IMGS_PER_CORE = B // NCORES            # 2
P = H * W                              # 65536 pixel-pairs per core
NQ = 4                                 # PE row-tile quarters
QP = P // NQ                           # 16384 pixels per quarter
RN = 512                               # pixels per strip-task
ROUNDS = QP // RN                      # 32 chunks per strip
NTASKS = NQ * ROUNDS                   # 128 matmul tasks
SPT = 60                               # weighted-sum subtiles per psum2 block
NSUB = NTASKS * 4                      # 512 subtiles of 128 pixels
NBLK = (NSUB + SPT - 1) // SPT         # 9 output blocks

# tuning knobs (env-overridable for experiments)
MM1_DT = os.environ.get("PALQ_MM1_DT", "bfloat16")   # bfloat16|float32|float32r
E_DT = os.environ.get("PALQ_E_DT", "float16")        # float16|float32|bfloat16
PALW_SPLIT = os.environ.get("PALQ_PALW_SPLIT", "0") == "1"  # hi/lo palW fix-up
MM1_SPLIT = os.environ.get("PALQ_MM1_SPLIT", "1") == "1"    # hi/lo x & palT


def _dt(name):
    return getattr(mybir.dt, name)


def build_bass(scale: float):
    nc = bacc.Bacc("TRN2", target_bir_lowering=False, debug=False)
    f32 = mybir.dt.float32
    e_dt = _dt(E_DT)
    mm1_dt = _dt(MM1_DT)

    # contraction rows per quarter: 6 = (rgb x 2 images); with hi/lo
    # split, 18 = [xh | xl | xh] against [ph | ph | pl]
    kr = 18 if MM1_SPLIT else 6
    xin = nc.dram_tensor("xin", [NQ, kr, QP], mm1_dt, kind="ExternalInput")
    palt = nc.dram_tensor("palt", [128, 128], mm1_dt, kind="ExternalInput")
    ebias = nc.dram_tensor("ebias", [128, 1], f32, kind="ExternalInput")
    palw_hi = nc.dram_tensor("palw_hi", [128, 8], e_dt, kind="ExternalInput")
    n_palw = 2 if (PALW_SPLIT and E_DT != "float32") else 1
    palw_lo = (
        nc.dram_tensor("palw_lo", [128, 8], e_dt, kind="ExternalInput")
        if n_palw == 2
        else None
    )
    out = nc.dram_tensor("out", [IMGS_PER_CORE, NBLK, 128, 3 * SPT], f32,
                         kind="ExternalOutput")

    with tile.TileContext(nc) as tc:
        import contextlib
        with contextlib.ExitStack() as ctx:
            singles = ctx.enter_context(tc.tile_pool(name="singles", bufs=1))
            epool = ctx.enter_context(tc.tile_pool(name="epool", bufs=4))
            ps1 = ctx.enter_context(tc.tile_pool(name="ps1", bufs=2, space="PSUM"))
            ps2 = ctx.enter_context(tc.tile_pool(name="ps2", bufs=2, space="PSUM"))
            vpool = ctx.enter_context(tc.tile_pool(name="vpool", bufs=2))
            opool = ctx.enter_context(tc.tile_pool(name="opool", bufs=3))

            # stationary palette (host pre-replicated into all 4 strips)
            # on the scalar HWDGE queue so it doesn't serialize with the
            # big pixel DMAs on the sync queue
            palt_sb = singles.tile([128, 128], mm1_dt)
            nc.scalar.dma_start(out=palt_sb, in_=palt.ap())

            # pre-warm the ACT exp table while input DMAs stream; reading
            # a framework-preloaded const avoids waiting on any engine
            warm = singles.tile([1, 1], f32)
            nc.scalar.activation(out=warm,
                                 in_=nc.const_aps.scalar_like(0.0, warm),
                                 func=mybir.ActivationFunctionType.Exp)

            # resident input pixels: quarter j on partitions [32j, 32j+kr);
            # chunk-major issue order, small first chunk, so round 0's
            # columns land as early as possible
            xsb = singles.tile([128, QP], mm1_dt)
            ebias_sb = singles.tile([128, 1], f32)
            # tiny, needed by the very first exp: goes first on sync
            nc.sync.dma_start(out=ebias_sb, in_=ebias.ap())
            palw_sb = singles.tile([128, 8], e_dt)
            if n_palw == 2:
                palw_lo_sb = singles.tile([128, 8], e_dt)
            bounds = [0, 512, 1536, 2560, 4096, 6144, 8192, 10240, 12288,
                      14336, QP]
            for h in range(len(bounds) - 1):
                sl = slice(bounds[h], bounds[h + 1])
                for j in range(NQ):
                    # first chunks split across both HWDGE queues so all
                    # four descriptors are generated concurrently
                    eng = nc.scalar if (h == 0 and j >= 2) else nc.sync
                    eng.dma_start(out=xsb[32 * j:32 * j + kr, sl],
                                  in_=xin.ap()[j, :, sl])
                if h == 0:
                    # needed by the first weighted-sum matmuls (~13us in)
                    nc.scalar.dma_start(out=palw_sb, in_=palw_hi.ap())
                    if n_palw == 2:
                        nc.scalar.dma_start(out=palw_lo_sb,
                                            in_=palw_lo.ap())

            # 128 matmul tasks i -> (strip j = i%4, chunk k = i//4), three
            # per 1536-col round (strips always distinct mod 4).  psum1 =
            # 2x3 banks, psum2 = 2x1 banks: exactly 8 PSUM banks.
            NRND = (NTASKS + 2) // 3               # 43 (last has 2 tasks)
            psum2 = None
            s = 0
            for r in range(NRND):
                tasks = [3 * r + m for m in range(3) if 3 * r + m < NTASKS]
                nt = len(tasks)
                psum1 = ps1.tile([128, 3 * RN], f32)
                for m, i in enumerate(tasks):
                    j, k = i % NQ, i // NQ
                    psl = slice(32 * j, 32 * j + kr)
                    nc.tensor.matmul(
                        out=psum1[:, RN * m:RN * (m + 1)],
                        lhsT=palt_sb[psl, :],
                        rhs=xsb[psl, RN * k:RN * (k + 1)],
                        start=True, stop=True,
                        tile_position=(32 * j, 0),
                    )
                e_sb = epool.tile([128, 3 * RN], e_dt)
                nc.scalar.activation(
                    out=e_sb[:, :RN * nt], in_=psum1[:, :RN * nt],
                    func=mybir.ActivationFunctionType.Exp,
                    scale=float(scale), bias=ebias_sb,
                )
                for t in range(4 * nt):
                    if psum2 is None:
                        psum2 = ps2.tile([128, 8 * SPT], f32)
                    u = s % SPT
                    nc.tensor.matmul(
                        out=psum2[:, 8 * u:8 * u + 8],
                        lhsT=e_sb[:, 128 * t:128 * (t + 1)],
                        rhs=palw_sb,
                        start=True, stop=(n_palw == 1),
                    )
                    if n_palw == 2:
                        nc.tensor.matmul(
                            out=psum2[:, 8 * u:8 * u + 8],
                            lhsT=e_sb[:, 128 * t:128 * (t + 1)],
                            rhs=palw_lo_sb,
                            start=False, stop=True,
                        )
                    s += 1
                    if s % SPT == 0 or s == NSUB or s == NSUB - 16:
                        # psum2 block (or last-block half) full: divide
                        # and ship out.  The final block flushes in two
                        # halves so the tail epilogue overlaps compute.
                        b = (s - 1) // SPT
                        v0 = 0 if s - SPT * b in (0, SPT) else (
                            (s - 1) % SPT + 1 - 16 if s == NSUB - 16
                            else NSUB - 16 - SPT * b)
                        v1 = (s - 1) % SPT + 1
                        nu = v1 - v0
                        psr = psum2[:, 8 * v0:8 * v1].rearrange(
                            "p (v e) -> p v e", e=8)
                        recA = vpool.tile([128, nu], f32, name="recA")
                        nc.vector.reciprocal(out=recA, in_=psr[:, :, 3])
                        recB = vpool.tile([128, nu], f32, name="recB")
                        nc.vector.reciprocal(out=recB, in_=psr[:, :, 7])
                        outA = opool.tile([128, 3 * nu], f32, name="outA")
                        outB = opool.tile([128, 3 * nu], f32, name="outB")
                        oA = outA.rearrange("p (v c) -> p v c", c=3)
                        oB = outB.rearrange("p (v c) -> p v c", c=3)
                        for c in range(3):
                            nc.vector.tensor_mul(out=oA[:, :, c],
                                                 in0=psr[:, :, c], in1=recA)
                            nc.vector.tensor_mul(out=oB[:, :, c],
                                                 in0=psr[:, :, 4 + c],
                                                 in1=recB)
                        osl = slice(3 * v0, 3 * v1)
                        # early blocks ride SWDGE so they don't delay input
                        # chunk descriptors; late blocks use the (by then
                        # idle) sync HWDGE queue, keeping the end-of-kernel
                        # GPSIMD drain empty (it costs ~2.6us when pending)
                        oeng = nc.gpsimd if b < 6 else nc.sync
                        oeng.dma_start(out=out.ap()[0, b, :, osl], in_=outA)
                        oeng.dma_start(out=out.ap()[1, b, :, osl], in_=outB)
                        if s % SPT == 0 or s == NSUB:
                            psum2 = None

    nc.compile()
    return nc


def _host_prep(images, palettes, scale):
    """Per-core input arrays. images [16,256,256,3] f32, palettes [16,64,3].
    scale = 2/temperature; the softmax logit is scale*dot + ebias."""
    import ml_dtypes

    imgs = np.ascontiguousarray(images, np.float32).reshape(B, P, C)
    pals = np.ascontiguousarray(palettes, np.float32)
    np_mm1 = {"float16": np.float16,
              "bfloat16": ml_dtypes.bfloat16}.get(MM1_DT, np.float32)
    np_e = {"float16": np.float16,
            "bfloat16": ml_dtypes.bfloat16}.get(E_DT, np.float32)
    in_maps = []
    for core in range(NCORES):
        ia, ib = imgs[2 * core], imgs[2 * core + 1]
        # per-quarter channel rows: [rgbA | rgbB] on the contraction dim
        x6 = np.empty((NQ, 6, QP), np.float32)
        x6[:, 0:3] = ia.reshape(NQ, QP, C).transpose(0, 2, 1)
        x6[:, 3:6] = ib.reshape(NQ, QP, C).transpose(0, 2, 1)

        pa, pb = pals[2 * core], pals[2 * core + 1]
        p6 = np.zeros((6, 128), np.float32)   # block-diag [pA^T | pB^T]
        p6[0:3, 0:64] = pa.T
        p6[3:6, 64:128] = pb.T

        if MM1_SPLIT:
            xh = x6.astype(np_mm1)
            xl = (x6 - xh.astype(np.float32)).astype(np_mm1)
            ph = p6.astype(np_mm1)
            pl = (p6 - ph.astype(np.float32)).astype(np_mm1)
            xin = np.concatenate([xh, xl, xh], axis=1)       # [NQ, 18, QP]
            palt_kr = np.concatenate([ph, ph, pl], axis=0)   # [18, 128]
        else:
            xin = x6.astype(np_mm1)
            palt_kr = p6.astype(np_mm1)
        kr = palt_kr.shape[0]
        palt = np.zeros((128, 128), palt_kr.dtype)           # strip-replicated
        for j in range(NQ):
            palt[32 * j:32 * j + kr] = palt_kr

        ebias = np.empty((128, 1), np.float32)
        ebias[0:64, 0] = -0.5 * scale * (pa * pa).sum(-1)
        ebias[64:128, 0] = -0.5 * scale * (pb * pb).sum(-1)

        palw = np.zeros((128, 8), np.float32)
        palw[0:64, 0:3] = pa
        palw[0:64, 3] = 1.0
        palw[64:128, 4:7] = pb
        palw[64:128, 7] = 1.0

        m = {"xin": xin, "palt": palt, "ebias": ebias}
        hi = palw.astype(np_e)
        m["palw_hi"] = hi
        if PALW_SPLIT and E_DT != "float32":
            m["palw_lo"] = (palw - hi.astype(np.float32)).astype(np_e)
        in_maps.append(m)
    return in_maps


def _host_post(results):
    """results[core]["out"] [2, 8, 128, 192] -> [16, 256, 256, 3]."""
    # subtile s -> (round r, t): s = 12r + t; task i = 3r + t//4 ->
    # (strip j = i%4, chunk k = i//4), q = t%4;
    # pixel = j*QP + k*512 + q*128 + row; stored at block s//SPT col s%SPT
    s_arr = np.arange(NSUB)
    r_arr = np.minimum(s_arr // 12, (NTASKS + 2) // 3 - 1)
    t_arr = s_arr - 12 * r_arr
    i_arr = 3 * r_arr + t_arr // 4
    base = (i_arr % NQ) * QP + (i_arr // NQ) * RN + (t_arr % 4) * 128
    out = np.empty((B, P, C), np.float32)
    for core in range(NCORES):
        o = results[core]["out"]
        dec = np.empty((IMGS_PER_CORE, P, C), np.float32)
        for s in range(NSUB):
            b, u = s // SPT, s % SPT
            dec[:, base[s]:base[s] + 128, :] = o[:, b, :, 3 * u:3 * u + 3]
        out[2 * core] = dec[0]
        out[2 * core + 1] = dec[1]
    return out.reshape(B, H, W, C)


_CACHE = {}


def _get_nc(scale: float):
    key = (round(float(scale), 12), MM1_DT, E_DT, PALW_SPLIT, MM1_SPLIT)
    if key not in _CACHE:
        _CACHE[key] = build_bass(scale)
    return _CACHE[key]


def kernel(images, palettes, temperature, _trace=False):
    scale = 2.0 / float(np.asarray(temperature))
    nc = _get_nc(scale)
    in_maps = _host_prep(images, palettes, scale)
    res = run_bass_kernel_spmd(nc, in_maps, core_ids=list(range(NCORES)),
                               trace=_trace)
    out = _host_post(res.results)
    if _trace:
        kernel.last_result = res
    return out
